# revision 22
# baseline (speedup 1.0000x reference)
"""Trainium2 Bass kernel: Llama-style attention block (prefill, start_pos=0).

Reference computation (per problem):
  q = x @ wq; k = x @ wk; v = x @ wv          (DIM=4096 -> 32 q-heads / 8 kv-heads, hd=128)
  rope(q, k) with interleaved (even, odd) pairs using freqs_cos/freqs_sin inputs
  scores = q @ k^T / sqrt(128) + mask ; p = softmax(scores) ; o = p @ v (GQA 4x)
  out = o @ wo
Distribution: tensor-parallel over heads on 8 cores. Core c owns q-heads
4c..4c+3 and kv-head c; wq/wk/wv sharded column-wise, wo row-wise. Each core
computes a full-shape partial of the output projection; the host sums the 8
partials (the row-parallel all-reduce, done at unshard time).

Layout strategy on-chip (per core):
  - host passes x transposed (xp, dim-major) so Q^T/K^T/V^T come out of the
    PE in [dims, seq] layout, exactly what attention needs (contraction over
    head_dim = partition axis).
  - RoPE: wq/wk columns are permuted on the host so each head's rotation
    pairs (even, odd) become (first 64, last 64) rows; the pair swap is a
    constant 128x128 matmul and the cos/sin combine is 3 DVE ops.
  - scores are computed transposed (S^T [k, q] blocks): exp runs on the
    scalar engine reading PSUM directly; P^T feeds P@V with no transposes.
  - everything is bf16 (PSUM accumulation fp32).
  - softmax denominators: pexp k-blocks pair-summed on the DVE, contracted
    with a ones-vector matmul per quad; diagonal blocks masked with a 0/1
    multiply and merged into one ones-matmul.
  - single x pass: all six projections (k, v, q0..q3) accumulate per x
    group (6 PSUM banks + 1 rotation spare + 1 aux), so x streams from HBM
    exactly once. This keeps the startup era compute-dense enough that the
    PE never starves against the HBM ramp (the old 2-pass layout re-read x
    and went HAM-cold twice at the start). Rope/transpose drains of chunk
    n-1 are spread between the x-groups of chunk n; chunk 3's drains spill
    into pass 2, interleaved with attention/out-projection work.
"""

import math

import numpy as np

import concourse.bass as bass
import concourse.mybir as mybir
import concourse.tile as tile
from concourse import bacc, bass_utils

DIM = 4096
N_HEADS = 32
N_KV = 8
HD = 128
SEQ = 2048
NCORES = 8
HPC = N_HEADS // NCORES          # q heads per core
QD = HPC * HD                    # 512 q-dims per core
SCALE = 1.0 / math.sqrt(HD)
NEG = -1.0e30

NQC = SEQ // 512                 # q chunks of 512
NKB = SEQ // 128                 # k blocks of 128
NKC = DIM // 128                 # contraction chunks of 128
XG = 4                           # kc chunks per x DMA group
NGRP = NKC // XG                 # 8 x-groups per chunk

F32 = mybir.dt.float32
BF = mybir.dt.bfloat16
EXP = mybir.ActivationFunctionType.Exp

_PROG_CACHE = {}


def _build_program(mask_mode: str):
    """mask_mode: 'causal' (skip upper blocks, multiplicative triangular
    diagonal mask), 'none' (no masking), 'full' (add arbitrary maskT)."""
    assert mask_mode in ("causal", "none", "full")
    nc = bacc.Bacc("TRN2", target_bir_lowering=False, debug=False,
                   num_devices=NCORES)

    # all operand tensors arrive pre-packed in SBUF layout (partition-major)
    # so every load is a fully-contiguous DMA with multi-KB lines
    xp = nc.dram_tensor("xp", [128, NQC, NGRP, XG, 512], BF,
                        kind="ExternalInput").ap()
    wq01 = nc.dram_tensor("wq01", [128, NKC, 256], BF,
                          kind="ExternalInput").ap()
    wq23 = nc.dram_tensor("wq23", [128, NKC, 256], BF,
                          kind="ExternalInput").ap()
    wk = nc.dram_tensor("wk", [128, NKC, HD], BF, kind="ExternalInput").ap()
    wv = nc.dram_tensor("wv", [128, NKC, HD], BF, kind="ExternalInput").ap()
    wo = nc.dram_tensor("wo", [128, HPC, DIM], BF, kind="ExternalInput").ap()
    cos2 = nc.dram_tensor("cos2", [HD, SEQ], BF, kind="ExternalInput").ap()
    sin2 = nc.dram_tensor("sin2", [HD, SEQ], BF, kind="ExternalInput").ap()
    rmat = nc.dram_tensor("rmat", [HD, HD], BF, kind="ExternalInput").ap()
    ident = nc.dram_tensor("ident", [128, 128], BF, kind="ExternalInput").ap()
    ones_col_d = nc.dram_tensor("ones_col", [128, 1], BF,
                                kind="ExternalInput").ap()
    ones_row_d = nc.dram_tensor("ones_row", [1, 128], BF,
                                kind="ExternalInput").ap()
    if mask_mode == "causal":
        trimask_d = nc.dram_tensor("trimask", [128, 128], BF,
                                   kind="ExternalInput").ap()
    if mask_mode == "full":
        maskT_d = nc.dram_tensor("maskT", [SEQ, SEQ], F32,
                                 kind="ExternalInput").ap()
    out = nc.dram_tensor("out", [SEQ, DIM], BF, kind="ExternalOutput").ap()

    with tile.TileContext(nc) as tc:
        with tc.tile_pool(name="persist", bufs=1) as pp:
            # ---- persistent tiles ----
            qt = [pp.tile([128, SEQ], BF, name=f"qt{h}") for h in range(HPC)]
            kt = pp.tile([128, SEQ], BF)
            vs = pp.tile([128, SEQ], BF)         # seq-major V, block i at cols i*128
            attn = [pp.tile([128, SEQ], BF, name=f"attn{h}")
                    for h in range(HPC)]
            rmat_sb = pp.tile([128, 128], BF)
            ident_sb = pp.tile([128, 128], BF)
            ones_sb = pp.tile([128, 1], BF)
            onesrow = pp.tile([1, 128], BF)
            nc.gpsimd.dma_start(ident_sb[:], ident[:])
            nc.gpsimd.dma_start(rmat_sb[:], rmat[:])
            nc.gpsimd.dma_start(ones_sb[:], ones_col_d[:])
            nc.gpsimd.dma_start(onesrow[:], ones_row_d[:])
            if mask_mode == "causal":
                trimask_sb = pp.tile([128, 128], BF)
                nc.gpsimd.dma_start(trimask_sb[:], trimask_d[:])
            cos_sb = pp.tile([128, SEQ], BF)
            sin_sb = pp.tile([128, SEQ], BF)
            wq01_sb = pp.tile([128, NKC, 256], BF)
            wq23_sb = pp.tile([128, NKC, 256], BF)
            wk_sb = pp.tile([128, NKC, HD], BF)
            wv_sb = pp.tile([128, NKC, HD], BF)
            wo_sb = pp.tile([128, HPC, DIM], BF)

            # startup weight loads, ordered by first use (kc slice s is
            # consumed at x half-group s/2 of chunk 0): fine slices first so
            # the very first matmuls only wait on ~256KB, then batches land
            # just-in-time while x streams on the scalar/gpsimd queues. wo
            # is deferred into chunk 1.
            for s, e in ((0, 2), (2, 4), (4, 8), (8, 16), (16, 24),
                         (24, 32)):
                nc.sync.dma_start(wk_sb[:, s:e, :], wk[:, s:e, :])
                nc.sync.dma_start(wv_sb[:, s:e, :], wv[:, s:e, :])
                nc.sync.dma_start(wq01_sb[:, s:e, :], wq01[:, s:e, :])
                nc.sync.dma_start(wq23_sb[:, s:e, :], wq23[:, s:e, :])

            # ================= pass 1: K, V, q0..q3 =================
            ps1 = tc.alloc_tile_pool(name="ps1", bufs=1, space="PSUM")
            # dummy matmuls keep the PE activity monitor warm through the
            # initial DMA window (else the first ~3.5us of real matmuls run
            # at half clock). The operand tile is memset on-device so the
            # warm-up does not wait on any startup DMA.
            wtile = pp.tile([128, 128], BF, name="warmsrc")
            nc.vector.memset(wtile[:], 0)
            warm = ps1.tile([128, 128], F32, tag="aux", bufs=1)
            for _ in range(40):
                nc.tensor.matmul(warm[:], wtile[:], wtile[:],
                                 start=True, stop=True)

            with tc.tile_pool(name="work", bufs=1) as wp:

                # drain order at each chunk boundary; acc allocation and MM
                # emission use the same order so PSUM slot releases pipeline
                # exactly against the boundary copies.
                ORDER = ("k", "q0", "q1", "q2", "q3", "v")

                def rope_drain(head_or_k, n, raw, psp=None):
                    # raw [128,512] bf16 SBUF (copied from the PSUM
                    # accumulator at the chunk boundary) -> rope -> qt/kt.
                    nsl = slice(n * 512, (n + 1) * 512)
                    dst = kt if head_or_k == "k" else qt[head_or_k]
                    if psp is None:
                        swp = ps1.tile([128, 512], F32, tag="aux", bufs=1,
                                       name=f"swp{head_or_k}_{n}")
                    else:
                        swp = psp.tile([128, 512], F32, tag="big", bufs=3,
                                       name=f"swp{head_or_k}_{n}")
                    nc.tensor.matmul(swp[:], rmat_sb[:], raw[:],
                                     start=True, stop=True)
                    nc.vector.tensor_mul(dst[:, nsl], swp[:], sin_sb[:, nsl])
                    tmp = wp.tile([128, 512], BF, tag="ropetmp", bufs=2,
                                  name=f"tmp{head_or_k}_{n}")
                    nc.vector.tensor_mul(tmp[:], raw[:], cos_sb[:, nsl])
                    nc.vector.tensor_add(dst[:, nsl], dst[:, nsl], tmp[:])

                def v_drain(n, raw, psp=None):
                    if psp is None:
                        vtr = ps1.tile([128, 512], BF, tag="aux", bufs=1,
                                       name=f"vtr_{n}")
                    else:
                        vtr = psp.tile([128, 512], BF, tag="big", bufs=3,
                                       name=f"vtr_{n}")
                    for b in range(4):
                        nc.tensor.transpose(vtr[:, b * 128:(b + 1) * 128],
                                            raw[:, b * 128:(b + 1) * 128],
                                            ident_sb[:])
                    nc.scalar.copy(vs[:, n * 512:(n + 1) * 512], vtr[:])

                def boundary_copies(n, accs):
                    # PSUM -> SBUF copies for all six accumulators, emitted
                    # back-to-back on alternating engines so the banks free
                    # in allocation order for the next chunk.
                    raws = {}
                    for i, name in enumerate(ORDER):
                        r = wp.tile([128, 512], BF, tag="raw", bufs=8,
                                    name=f"raw_{name}_{n}")
                        if i % 2 == 0:
                            nc.scalar.copy(r[:], accs[name][:])
                        else:
                            nc.vector.tensor_copy(r[:], accs[name][:])
                        raws[name] = r
                    return raws

                def drain_piece(i, n, raws, psp=None):
                    name = ORDER[i]
                    if name == "k":
                        rope_drain("k", n, raws["k"], psp)
                    elif name == "v":
                        v_drain(n, raws["v"], psp)
                    else:
                        rope_drain(int(name[1]), n, raws[name], psp)

                # ---- pass 1 main loop: 6 accumulators per chunk ----
                # the rope/transpose PE work of chunk n-1 is spread between
                # the x-groups of chunk n (one piece per group) so the aux
                # PSUM bank never serializes against the PE stream.
                pend = None              # raw copies of chunk n-1
                for n in range(NQC):
                    accs = {name: ps1.tile([128, 512], F32, tag="acc",
                                           bufs=7, name=f"acc_{name}_{n}")
                            for name in ORDER}
                    # chunk 0 uses half-size groups fanned over two DMA
                    # queues so the startup pipeline fills sooner
                    ng_n = 2 * NGRP if n == 0 else NGRP
                    for g in range(ng_n):
                        xg = wp.tile([128, XG, 512], BF, tag="xg", bufs=8,
                                     name=f"xg1_{n}_{g}")
                        eng = (nc.scalar, nc.gpsimd)[g % 2]
                        if n == 0:
                            eng.dma_start(
                                xg[:, 0:2, :],
                                xp[:, 0, g // 2, (g % 2) * 2:(g % 2) * 2 + 2, :])
                            kks = range(2)
                        else:
                            eng.dma_start(xg[:], xp[:, n, g, :, :])
                            kks = range(XG)
                        for kk in kks:
                            kc = g * (2 if n == 0 else XG) + kk
                            st, sp = (kc == 0), (kc == NKC - 1)
                            nc.tensor.matmul(accs["k"][:], wk_sb[:, kc, :],
                                             xg[:, kk, :], start=st, stop=sp)
                            nc.tensor.matmul(accs["q0"][:],
                                             wq01_sb[:, kc, 0:128],
                                             xg[:, kk, :], start=st, stop=sp)
                            nc.tensor.matmul(accs["q1"][:],
                                             wq01_sb[:, kc, 128:256],
                                             xg[:, kk, :], start=st, stop=sp)
                            nc.tensor.matmul(accs["q2"][:],
                                             wq23_sb[:, kc, 0:128],
                                             xg[:, kk, :], start=st, stop=sp)
                            nc.tensor.matmul(accs["q3"][:],
                                             wq23_sb[:, kc, 128:256],
                                             xg[:, kk, :], start=st, stop=sp)
                            nc.tensor.matmul(accs["v"][:], wv_sb[:, kc, :],
                                             xg[:, kk, :], start=st, stop=sp)
                        if n == 0 and g == 1:
                            # cos/sin land behind the first x groups, just
                            # before their first reader (chunk-0 drains)
                            nc.sync.dma_start(cos_sb[:], cos2[:])
                            nc.sync.dma_start(sin_sb[:], sin2[:])
                        if n == 1 and g == 0:
                            # wo streams during chunk 1: first needed in
                            # pass 2, well after all x / weights
                            for gg in range(2):
                                nc.sync.dma_start(
                                    wo_sb[:, :, gg * 2048:(gg + 1) * 2048],
                                    wo[:, :, gg * 2048:(gg + 1) * 2048])
                        # rope/transpose of the previous chunk, one piece
                        # per x-group
                        if pend is not None and 1 <= g <= 6:
                            drain_piece(g - 1, n - 1, pend)
                    pend = boundary_copies(n, accs)

                ps1.release()

                # ========== pass 2: attention + out-projection ==========
                psp = tc.alloc_tile_pool(name="ps2", bufs=1, space="PSUM")
                n3, raws3 = NQC - 1, pend

                def a_head_items(jj, h):
                    """Attention for chunk jj, head h, as a list of PE-work
                    closures in emission order: score matmul + exp per
                    k-block, pv matmuls lagging 3 blocks behind (so each
                    exp has the latency of several interleaved items), one
                    merged denominator matmul (quads accumulate on the DVE),
                    reciprocal, and the final normalization. The caller
                    interleaves these items with out-projection work so the
                    scalar engine's exp stream never stalls the PE."""
                    jsl = slice(jj * 512, (jj + 1) * 512)
                    nblk = 4 * jj + 4 if mask_mode == "causal" else NKB
                    pv = psp.tile([128, 512], F32, tag="pv", bufs=1,
                                  name=f"pv{h}_{jj}")
                    dn = psp.tile([1, 512], F32, tag="dn", bufs=1,
                                  name=f"dn{h}_{jj}")
                    state = {"pending": None, "ppend": None, "sum": None,
                             "diags": [], "rcpr": None}
                    fl = []

                    def add_sum(t):
                        # running denominator operand, accumulated on DVE
                        if state["sum"] is None:
                            state["sum"] = t
                        else:
                            nc.vector.tensor_add(state["sum"][:],
                                                 state["sum"][:], t[:])

                    def flush_one():
                        i, pexp, off, diag = fl.pop(0)
                        nc.tensor.matmul(pv[:, off:],
                                         vs[:, i * 128:(i + 1) * 128],
                                         pexp[:, off:],
                                         start=(i == 0),
                                         stop=(i == nblk - 1))
                        if diag:
                            state["diags"].append((pexp, off))
                            if len(state["diags"]) == 4:
                                p0 = state["diags"][0][0]
                                for pe_i, off_i in state["diags"][1:]:
                                    nc.vector.tensor_add(p0[:, off_i:],
                                                         p0[:, off_i:],
                                                         pe_i[:, off_i:])
                                add_sum(p0)
                        elif state["pending"] is None:
                            state["pending"] = pexp
                        else:
                            pr = wp.tile([128, 512], BF, tag="ppair", bufs=3,
                                         name=f"pr{h}_{jj}_{i}")
                            nc.vector.tensor_add(pr[:], state["pending"][:],
                                                 pexp[:])
                            state["pending"] = None
                            if state["ppend"] is None:
                                state["ppend"] = pr
                            else:
                                qd = wp.tile([128, 512], BF, tag="quad",
                                             bufs=2, name=f"qd{h}_{jj}_{i}")
                                nc.vector.tensor_add(qd[:], state["ppend"][:],
                                                     pr[:])
                                state["ppend"] = None
                                add_sum(qd)

                    def emit_block(i):
                        def go():
                            r = i - 4 * jj
                            diag = mask_mode == "causal" and r >= 0
                            off = 128 * r if (diag and r > 0) else 0
                            stp = psp.tile([128, 512], F32, tag="stp",
                                           bufs=3, name=f"st{h}_{jj}_{i}")
                            nc.tensor.matmul(
                                stp[:, off:], kt[:, i * 128:(i + 1) * 128],
                                qt[h][:, jj * 512 + off:(jj + 1) * 512],
                                start=True, stop=True)
                            if mask_mode == "full":
                                mt = wp.tile([128, 512], F32, tag="mt",
                                             bufs=3)
                                nc.sync.dma_start(
                                    mt[:],
                                    maskT_d[i * 128:(i + 1) * 128, jsl])
                                nc.vector.tensor_add(stp[:], stp[:], mt[:])
                            pexp = wp.tile([128, 512], BF, tag="pexp",
                                           bufs=8, name=f"pexp{h}_{jj}_{i}")
                            nc.scalar.activation(pexp[:, off:], stp[:, off:],
                                                 EXP, scale=SCALE)
                            if diag:
                                nc.vector.tensor_mul(pexp[:, off:off + 128],
                                                     pexp[:, off:off + 128],
                                                     trimask_sb[:])
                            fl.append((i, pexp, off, diag))
                        return go

                    def final_dn():
                        assert state["pending"] is None
                        assert state["ppend"] is None
                        nc.tensor.matmul(dn[:], ones_sb[:], state["sum"][:],
                                         start=True, stop=True)

                    def rcp_item():
                        rcp = wp.tile([1, 512], F32, tag="rcp", bufs=2)
                        nc.vector.reciprocal_approx_fast(rcp[:], dn[:])
                        rcpr = wp.tile([1, 512], BF, tag="rcpr", bufs=2)
                        nc.vector.tensor_copy(rcpr[:], rcp[:])
                        state["rcpr"] = rcpr

                    def finalize():
                        bcs = wp.tile([128, 512], BF, tag="bcs", bufs=2)
                        nc.gpsimd.partition_broadcast(bcs[:],
                                                      state["rcpr"][:])
                        nc.vector.tensor_mul(attn[h][:, jsl], pv[:], bcs[:])

                    items = []
                    for i in range(nblk):
                        items.append(emit_block(i))
                        if i >= 3:
                            items.append(flush_one)
                    items.extend([flush_one] * min(3, nblk))
                    items.extend([final_dn, rcp_item, finalize])
                    return items

                def wo_items(m, eng_flip, n_act=2):
                    """One 128-row seq block of the output projection as a
                    list of PE-work closures (one per stationary operand).
                    n_act of the 8 PSUM drains go to the scalar engine, the
                    rest to the DVE — tuned per unit to whichever engine
                    the paired attention head loads less."""
                    msl = slice(m * 128, (m + 1) * 128)
                    acts = {(round(k * 8 / n_act) + eng_flip) % 8
                            for k in range(n_act)}
                    items = []
                    unit = {}

                    def start_w4(w4):
                        def go():
                            unit[w4] = [
                                psp.tile([128, 512], F32, tag="big", bufs=3,
                                         name=f"yp{m}_{w4}_{i}")
                                for i in range(2)]
                        return go

                    def mm(w4, kc):
                        def go():
                            for i in range(2):
                                ncol = w4 * 2 + i
                                nc.tensor.matmul(
                                    unit[w4][i][:], attn[kc][:, msl],
                                    wo_sb[:, kc,
                                          ncol * 512:(ncol + 1) * 512],
                                    start=(kc == 0), stop=(kc == HPC - 1))
                        return go

                    def drain_w4(w4):
                        def go():
                            for i in range(2):
                                ncol = w4 * 2 + i
                                ysb = wp.tile([128, 512], BF, tag="ysb",
                                              bufs=6)
                                if ncol in acts:
                                    nc.scalar.copy(ysb[:], unit[w4][i][:])
                                else:
                                    nc.vector.tensor_copy(ysb[:],
                                                          unit[w4][i][:])
                                nc.sync.dma_start(
                                    out[msl, ncol * 512:(ncol + 1) * 512],
                                    ysb[:])
                        return go

                    for w4 in range(4):
                        items.append(start_w4(w4))
                        for kc in range(HPC):
                            items.append(mm(w4, kc))
                        items.append(drain_w4(w4))
                    return items

                def interleave(*streams, bias=None):
                    """Bresenham-merge item streams and emit them: at every
                    step, run the item from the stream that is furthest
                    behind proportionally. A stream with bias b < 1
                    exhausts when the others are at fraction b (i.e. it
                    finishes early)."""
                    streams = [s for s in streams if s]
                    bias = bias or [1.0] * len(streams)
                    pos = [0] * len(streams)
                    while True:
                        best, bf = None, None
                        for si, s in enumerate(streams):
                            if pos[si] < len(s):
                                frac = pos[si] * bias[si] / len(s)
                                if best is None or frac < bf:
                                    best, bf = si, frac
                        if best is None:
                            return
                        streams[best][pos[best]]()
                        pos[best] += 1

                # ---- pass 2 main loop ----
                # ascending chunks, out-projection of chunk j-1 interleaved
                # item-by-item with attention of chunk j: every unit pairs
                # the ACT-heavy attention (exp per k-block) with the
                # ACT-free wo matmuls so the exp stream never stalls the
                # PE. The chunk-3 rope/transpose pieces ride where the PE
                # has filler.
                # chunk-3 rope/transpose pieces: only kt and qt[0] ride the
                # (DVE-tight) entry; the rest spread over j=1/2 units,
                # still far ahead of their first readers in the j=3 units.
                piece_slots = {(0, 0): 0, (0, 2): 1, (1, 0): 5, (1, 1): 2,
                               (2, 0): 3, (2, 1): 4}
                for h in range(HPC):
                    if (0, h) in piece_slots:
                        drain_piece(piece_slots[0, h], n3, raws3, psp)
                    interleave(a_head_items(0, h))
                for j in range(1, NQC):
                    for h in range(HPC):
                        if (j, h) in piece_slots:
                            drain_piece(piece_slots[j, h], n3, raws3, psp)
                        last = j == NQC - 1 and h == HPC - 1
                        interleave(a_head_items(j, h),
                                   wo_items(4 * (j - 1) + h, h,
                                            n_act=5 - j),
                                   bias=[0.7 if last else 0.85, 1.0])
                # ---- tail: out-proj for chunk 3 ----
                for m in range(12, 16):
                    interleave(wo_items(m, m, n_act=4))
                psp.release()

    nc.compile()
    return nc


def get_program(mask_mode: str):
    if mask_mode not in _PROG_CACHE:
        _PROG_CACHE[mask_mode] = _build_program(mask_mode)
    return _PROG_CACHE[mask_mode]


# ====================== host-side preparation ======================

_PERM128 = np.concatenate([np.arange(0, 128, 2), np.arange(1, 128, 2)])


def _bf16(a: np.ndarray) -> np.ndarray:
    import ml_dtypes
    return np.ascontiguousarray(a.astype(np.float32).astype(ml_dtypes.bfloat16))


def _perm_cols(w: np.ndarray, n_heads: int) -> np.ndarray:
    """Permute each head's 128 columns: even dims first, odd dims last."""
    cols = np.concatenate([h * 128 + _PERM128 for h in range(n_heads)])
    return w[:, cols]


def _classify_mask(mask: np.ndarray) -> str:
    if not np.any(mask):
        return "none"
    iu = np.triu_indices(SEQ, 1)
    upper = mask[iu]
    lower_ok = not np.any(np.tril(mask))
    upper_ok = bool(np.all(np.isneginf(upper) | (upper <= -1e9)))
    if lower_ok and upper_ok:
        return "causal"
    return "full"


def _pack_w(w: np.ndarray) -> np.ndarray:
    """[DIM, M] -> SBUF layout [128, NKC, M] (partition-major, contiguous)."""
    m = w.shape[1]
    return _bf16(w.reshape(NKC, 128, m).transpose(1, 0, 2))


def _host_inputs(x, wq, wk, wv, wo, freqs_cos, freqs_sin, mask):
    wq_p = _perm_cols(np.asarray(wq, np.float32), N_HEADS)
    wk_p = _perm_cols(np.asarray(wk, np.float32), N_KV)
    wv_ = np.asarray(wv, np.float32)
    wo_ = np.asarray(wo, np.float32)

    # x packed to [128, NQC, NGRP, XG, 512]: xp[p,n,g,kk,s] = x[n*512+s,
    # (g*XG+kk)*128+p] — every DMA group is one contiguous multi-KB run.
    x2 = np.asarray(x, np.float32).reshape(SEQ, DIM)
    xpk = _bf16(x2.reshape(NQC, 512, NGRP, XG, 128)
                .transpose(4, 0, 2, 3, 1))

    cosT = np.asarray(freqs_cos, np.float32).T              # [64, SEQ]
    sinT = np.asarray(freqs_sin, np.float32).T
    cos2 = np.concatenate([cosT, cosT], 0)                  # [128, SEQ]
    sin2 = np.concatenate([sinT, sinT], 0)

    rmat = np.zeros((HD, HD), np.float32)
    rmat[np.arange(64) + 64, np.arange(64)] = -1.0   # swp[:64] = -raw[64:]
    rmat[np.arange(64), np.arange(64) + 64] = 1.0    # swp[64:] = raw[:64]
    ident = np.eye(128, dtype=np.float32)

    mask = np.asarray(mask, np.float32)
    mode = _classify_mask(mask)

    common = {"xp": xpk, "cos2": _bf16(cos2), "sin2": _bf16(sin2),
              "rmat": _bf16(rmat), "ident": _bf16(ident),
              "ones_col": _bf16(np.ones((HD, 1), np.float32)),
              "ones_row": _bf16(np.ones((1, HD), np.float32))}
    if mode == "causal":
        kk = np.arange(128)[:, None]
        qq = np.arange(128)[None, :]
        common["trimask"] = _bf16((kk <= qq).astype(np.float32))
    elif mode == "full":
        m = np.where(np.isneginf(mask), NEG, mask)
        common["maskT"] = np.ascontiguousarray(m.T)

    in_maps = []
    for c in range(NCORES):
        im = dict(common)
        wq_c = wq_p[:, c * QD:(c + 1) * QD]
        im["wq01"] = _pack_w(wq_c[:, 0:256])
        im["wq23"] = _pack_w(wq_c[:, 256:512])
        im["wk"] = _pack_w(wk_p[:, c * HD:(c + 1) * HD])
        im["wv"] = _pack_w(wv_[:, c * HD:(c + 1) * HD])
        # wo packed to [128, HPC, DIM]: wo[p,kc,nn] = wo_[kc*128+p, nn]
        im["wo"] = _bf16(wo_[c * QD:(c + 1) * QD, :]
                         .reshape(HPC, 128, DIM).transpose(1, 0, 2))
        in_maps.append(im)
    return mode, in_maps


def _scores_safe(x, wq, wk):
    """The device softmax skips the max-subtraction (scores from
    setup_inputs()-scaled weights are O(5), so exp() is exact and safe).
    Estimate the score magnitude; if exp could overflow fp32, fall back."""
    sx = float(np.sqrt(np.mean(np.square(x), dtype=np.float64)))
    sq = sx * float(np.sqrt(np.mean(np.square(wq), dtype=np.float64)) * np.sqrt(DIM))
    sk = sx * float(np.sqrt(np.mean(np.square(wk), dtype=np.float64)) * np.sqrt(DIM))
    # rope with arbitrary freqs can scale q/k by ~sqrt(2); 7 sigma tail margin
    return 2.0 * sq * sk * 7.0 < 80.0


def _numpy_fallback(x, wq, wk, wv, wo, freqs_cos, freqs_sin, mask):
    """Slow but numerically-safe host path (stable softmax), used only when
    the score magnitudes could overflow the device's unshifted exp."""
    x2 = x.reshape(SEQ, DIM).astype(np.float64)
    q = (x2 @ wq.astype(np.float64)).reshape(SEQ, N_HEADS, HD)
    k = (x2 @ wk.astype(np.float64)).reshape(SEQ, N_KV, HD)
    v = (x2 @ wv.astype(np.float64)).reshape(SEQ, N_KV, HD)
    cos = freqs_cos.astype(np.float64)[:, None, :]
    sin = freqs_sin.astype(np.float64)[:, None, :]

    def rope(t):
        a, b = t[..., 0::2], t[..., 1::2]
        out = np.empty_like(t)
        out[..., 0::2] = a * cos - b * sin
        out[..., 1::2] = a * sin + b * cos
        return out

    q, k = rope(q), rope(k)
    m64 = mask.astype(np.float64)
    outh = np.empty((SEQ, N_HEADS, HD))
    for h in range(N_HEADS):
        g = h // (N_HEADS // N_KV)
        s = q[:, h, :] @ k[:, g, :].T / math.sqrt(HD) + m64
        p = np.exp(s - s.max(-1, keepdims=True))
        p /= p.sum(-1, keepdims=True)
        outh[:, h, :] = p @ v[:, g, :]
    y = outh.reshape(SEQ, N_HEADS * HD) @ wo.astype(np.float64)
    return y.astype(np.float32).reshape(1, SEQ, DIM)


def kernel(x, wq, wk, wv, wo, freqs_cos, freqs_sin, mask, cache_k, cache_v,
           start_pos, **_unused):
    sp = int(np.asarray(start_pos))
    x = np.asarray(x, np.float32)
    wq = np.asarray(wq, np.float32)
    wk = np.asarray(wk, np.float32)
    wv = np.asarray(wv, np.float32)
    wo = np.asarray(wo, np.float32)
    mask = np.asarray(mask, np.float32)
    if sp != 0:
        raise NotImplementedError("kernel assumes start_pos == 0 prefill")
    if not _scores_safe(x, wq, wk):
        return _numpy_fallback(x, wq, wk, wv, wo,
                               np.asarray(freqs_cos, np.float32),
                               np.asarray(freqs_sin, np.float32), mask)

    mode, in_maps = _host_inputs(x, wq, wk, wv, wo,
                                 freqs_cos, freqs_sin, mask)
    nc = get_program(mode)
    res = bass_utils.run_bass_kernel_spmd(nc, in_maps,
                                          core_ids=list(range(NCORES)))
    acc = np.zeros((SEQ, DIM), np.float64)
    for r in res.results:
        acc += r["out"].astype(np.float64)
    return acc.astype(np.float32).reshape(1, SEQ, DIM)
